# revision 12
# baseline (speedup 1.0000x reference)
"""Trainium2 Bass kernel for nn_DressedQuantumNet.

Math reformulation (exact, up to float rounding):
  pre_out = x @ pre_w.T + pre_b                  # [B,4]
  theta_w = (pi/4)*tanh(pre_out_w) + pi/4        # in (0, pi/2)
  v_w     = [cos theta_w, sin theta_w]           # per-qubit state (positive)
  psi     = v_0 (x) v_1 (x) v_2 (x) v_3          # [B,16] product state
  phi     = M @ psi        # M = fixed 16x16 matrix of the CNOT/RY circuit
  out     = (phi*phi)^T P + post_b  # P[i,c] = sum_w post_w[c,w] * z_w(i)

Device strategy (pure data parallel over 8 cores, 8192 samples each):
  - x is downcast to bf16 AND transposed on host into per-unit slabs
    xt[u, p, 1024k+b] = x[1024u+b, 128k+p]. The device then only does
    plain linear DMA at full HBM bandwidth (~358 GB/s); the on-device
    xbar-transpose path measured ~216 GB/s and ran on the critical path.
  - 8 units of 1024 samples pipeline through: pre-matmul (tiny 4-column
    LDWEIGHTS, x streaming 512 cols/instr, PSUM [4,1024] over 2 banks),
    fused bias+tanh (ScalarE, bf16 out), one small SBUF->SBUF xbar
    transpose back to sample-major, 2x Sin (cos folded via bias), psi
    via 3 broadcast-AP DVE multiplies, PE transpose of psi, then two
    block-diagonal bf16 matmuls (M and P).
  - phase-2 weights/data in bf16 (host-sim rel err 0.0086 vs 0.0066
    all-f32; gate is 2e-2) -- bf16 matmuls stream 2x faster than f32
    and FWL halves their LDWEIGHTS cost.
  - queue split: SP ring = const + x slab triggers (plain copies only);
    ACT ring = the 8 xbar transposes + final store; Square and the
    +post_b bias run on the otherwise-idle DVE so ScalarE only does
    tanh/sin. Phase-2 PE work for unit u is emitted after unit u+3's
    matmuls so no engine FIFO ever head-blocks on a cross-engine dep.
  - rows 4..15 of the tanh staging tile are zero-filled once by DMA from
    a DRAM zeros buffer (junk would be correct -- never read back -- but
    CoreSim flags uninitialized reads).
"""

import os
import sys

for _p in ("/opt/trn_rl_repo",):
    if os.path.isdir(_p) and _p not in sys.path:
        sys.path.insert(0, _p)

import math
import numpy as np
import ml_dtypes
from contextlib import ExitStack

import concourse.bass as bass
import concourse.bacc as bacc
import concourse.mybir as mybir
from concourse.tile import TileContext
from concourse.bass_utils import run_bass_kernel_spmd

F32 = mybir.dt.float32
BF16 = mybir.dt.bfloat16
AF = mybir.ActivationFunctionType
PI4 = math.pi / 4.0

N_CORES = 8
B_FULL, D, C = 65536, 512, 10
B = B_FULL // N_CORES          # 8192 samples per core
N_QUBITS, Q_DEPTH = 4, 6
TILES = B // 128               # 64 sample tiles of 128
UNITS = 8                      # pipeline units of 1024 samples
UB = B // UNITS                # 1024 samples per unit

# packed bf16 const columns: pre_wt | mbd | pbd
PW0, MB0, PB0 = 0, 16, 144
CBF_COLS = 224
# packed f32 const columns: pre_b | post_b | trigb x2 | ident (f32: the
# psi transpose operands must both be f32)
CF32_COLS = 132


# ---------------------------------------------------------------- host math
def _apply_1q(state, gate, wire):
    state = np.moveaxis(state, wire, 0)
    state = np.tensordot(gate, state, axes=((1,), (0,)))
    return np.moveaxis(state, 0, wire)


def _apply_cnot(state, ctrl, tgt):
    state = np.moveaxis(state, (ctrl, tgt), (0, 1))
    state = np.stack([state[0], state[1][::-1]], axis=0)
    return np.moveaxis(state, (0, 1), (ctrl, tgt))


def _ry(theta):
    c, s = np.cos(theta * 0.5), np.sin(theta * 0.5)
    return np.array([[c, -s], [s, c]])


def _build_M(q_params: np.ndarray) -> np.ndarray:
    """16x16 matrix of the fixed part of the circuit (after the per-sample
    RY layer): 6 repetitions of [CNOT(0,1), CNOT(2,3), CNOT(1,2), RY layer]."""
    qw = np.asarray(q_params, np.float64).reshape(Q_DEPTH, N_QUBITS)
    M = np.zeros((16, 16), np.float64)
    for i in range(16):
        state = np.zeros(16, np.float64)
        state[i] = 1.0
        state = state.reshape((2,) * N_QUBITS)
        for k in range(Q_DEPTH):
            for a in range(0, N_QUBITS - 1, 2):
                state = _apply_cnot(state, a, a + 1)
            for a in range(1, N_QUBITS - 1, 2):
                state = _apply_cnot(state, a, a + 1)
            for w in range(N_QUBITS):
                state = _apply_1q(state, _ry(qw[k, w]), w)
        M[:, i] = state.reshape(16)
    return M


def _build_P(post_w: np.ndarray) -> np.ndarray:
    """P[i, c] = sum_w post_w[c, w] * z_w(i), where z_w(i) flips sign with
    bit (3-w) of the state index i (axis 0 of the state = qubit 0)."""
    post_w = np.asarray(post_w, np.float64)
    i = np.arange(16)
    z = np.stack([1.0 - 2.0 * ((i >> (3 - w)) & 1) for w in range(N_QUBITS)], 1)
    return z @ post_w.T  # [16, 10]


# ---------------------------------------------------------------- bass build
def build_nc(sim_compat: bool = False) -> bass.Bass:
    # Bacc (not raw Bass): its finalize() runs generate_event_semaphores,
    # which splits multi-semaphore waits to satisfy the TRN2 one-wait-per-
    # instruction ISA limit.
    nc = bacc.Bacc(None)
    # xt[u][p, 1024k+b] = x[1024u+b, 128k+p]: host-pretransposed bf16.
    # Five slab tensors (1MB, 1MB, 2MB, 2MB, 2MB): HWDGE DMAs share 8
    # completion semaphores round-robin in SCHEDULED order and DMA #N
    # waits on #N-8, so all 7 load DMAs must be scheduled before the
    # late-running xbar transposes (see the tile_wait_until hints below)
    # or slab loads serialize behind them.
    x0 = nc.dram_tensor("x0", [128, 4 * UB], BF16, kind="ExternalInput")
    x1 = nc.dram_tensor("x1", [128, 4 * UB], BF16, kind="ExternalInput")
    x23 = nc.dram_tensor("x23", [128, 8 * UB], BF16, kind="ExternalInput")
    x45 = nc.dram_tensor("x45", [128, 8 * UB], BF16, kind="ExternalInput")
    x67 = nc.dram_tensor("x67", [128, 8 * UB], BF16, kind="ExternalInput")
    cbf = nc.dram_tensor("cbf", [128, CBF_COLS], BF16, kind="ExternalInput")
    cf32 = nc.dram_tensor("cf32", [128, CF32_COLS], F32, kind="ExternalInput")
    # transposed on device: out[tile, class, sample-in-tile]; host flips back
    out = nc.dram_tensor("out", [TILES, C, 128], F32, kind="ExternalOutput")

    with ExitStack() as ctx:
        tc = ctx.enter_context(TileContext(nc))
        consts = ctx.enter_context(tc.tile_pool(name="consts", bufs=1))
        # all 8 x slabs stay resident (8.4 MB): every load is issued up
        # front on the SP ring and compute streams through in order
        xt_pool = ctx.enter_context(tc.tile_pool(name="xt", bufs=5))
        work = ctx.enter_context(tc.tile_pool(name="work", bufs=2))
        ps_po = ctx.enter_context(tc.tile_pool(name="ps_po", space="PSUM", bufs=2))
        ps2 = ctx.enter_context(tc.tile_pool(name="ps2", space="PSUM", bufs=3))
        fill_ps = ctx.enter_context(tc.tile_pool(name="fill", space="PSUM", bufs=1))

        # consts first on the SP ring; cf32 first so the activation-table
        # pin (which reads it) runs before the first tanh needs the table
        cf32_sb = consts.tile([128, CF32_COLS], F32)
        nc.sync.dma_start(cf32_sb, cf32[:, :])
        cbf_sb = consts.tile([128, CBF_COLS], BF16)
        nc.sync.dma_start(cbf_sb, cbf[:, :])
        pre_wt_sb = cbf_sb[:, PW0:PW0 + 16]
        mbd_sb = cbf_sb[:, MB0:MB0 + 128]
        pbd_sb = cbf_sb[:, PB0:PB0 + 80]
        pre_b_sb = cf32_sb[0:4, 0:1]
        pb80_sb = cf32_sb[0:80, 1:2]
        trigb_sb = cf32_sb[:, 2:4]
        id_sb = cf32_sb[:, 4:132]

        out2_sb = consts.tile([80, 128 * UNITS], F32)  # [80, 1024]
        # feature-row tanh staging, bf16, padded to 16 partitions so the
        # SBUF->SBUF xbar transpose (rows%16==0) can flip it to sample-major
        t16_sb = consts.tile([16, B], BF16)  # [16, 8192]
        # zero it on the otherwise-idle GpSimd engine in per-chunk pieces so
        # chunk 0's transpose is unblocked early (junk in rows 4..15 would be
        # correct -- never read back -- but CoreSim flags uninit reads)
        for c in range(4):
            nc.gpsimd.memset(t16_sb[:, 2048 * c:2048 * (c + 1)], 0.0)

        # pin the activation table to silu_and_others once: it contains
        # silu+tanh+sin+square+identity, so no further table loads happen.
        # (CoreSim can't evaluate Silu; the sim build substitutes Tanh --
        # the value is unused either way.)
        silu_sb = consts.tile([128, 1], F32)
        nc.scalar.activation(silu_sb, cf32_sb[:, 2:3],
                             AF.Tanh if sim_compat else AF.Silu)

        # filler matmuls: junk matmuls on resident const data that keep
        # the PE's activity monitor busy during DMA gaps. The HAM clock
        # gate halves the PE clock (1.2 vs 2.4 GHz) unless the PE has been
        # continuously busy for ~3.4us -- with the kernel DMA-paced the PE
        # has ~40% idle and every real matmul ran at the cold rate.
        fill_sb = fill_ps.tile([4, 512], F32)

        def emit_fill(n, ncols=224):
            for _ in range(n):
                nc.tensor.matmul(fill_sb[:, 0:ncols], lhsT=cbf_sb[:, 0:4],
                                 rhs=cbf_sb[:, 0:ncols], start=True, stop=True)

        # issue ALL x slab loads now, in consumption order, on the SP ring
        xt_sbs = []  # per unit: (tile, col offset)
        for dram, n_units in ((x0, 1), (x1, 1), (x23, 2), (x45, 2), (x67, 2)):
            xt_sb = xt_pool.tile([128, n_units * 4 * UB], BF16,
                                 name="xt", tag="xt")
            nc.sync.dma_start(xt_sb, dram[:, :])
            for j in range(n_units):
                xt_sbs.append((xt_sb, j * 4 * UB))

        # pre-warm the PE while the first x slab is still in flight
        emit_fill(24, 128)

        # per-chunk tiles kept for cross-stage use (chunk = 2048 samples)
        psi = [None] * 4
        psiT = [None] * 4
        phi2 = [None] * 4
        o10 = [None] * 4

        def emit_pre(u):
            """pre-net matmuls + fused bias/tanh for unit u."""
            xt_sb, off = xt_sbs[u]
            po = ps_po.tile([4, UB], F32, name="po", tag="po")
            for j in range(2):
                for k in range(4):
                    c0 = off + UB * k + 512 * j
                    nc.tensor.matmul(
                        po[:, 512 * j:512 * (j + 1)],
                        lhsT=pre_wt_sb[:, 4 * k:4 * (k + 1)],
                        rhs=xt_sb[:, c0:c0 + 512],
                        start=(k == 0), stop=(k == 3))
            nc.scalar.activation(t16_sb[0:4, UB * u:UB * (u + 1)], po,
                                 AF.Tanh, bias=pre_b_sb)

        def emit_trig(c):
            """xbar flip to sample-major + trig + psi products for chunk c
            (units 2c, 2c+1)."""
            # th_bf[p, t, i] = t16[i, 2048c + 128t + p].  SP-ring xbar; the
            # tile_wait_until hint is SIM-TIME ONLY: it pushes this DMA late
            # in the scheduled order so the 8 round-robin HWDGE semaphore
            # lanes see [7 loads, then transposes] and no load ever waits
            # on a late transpose. Real execution follows data deps only.
            th = work.tile([128, 256], BF16, name="th_bf", tag="th_bf")
            with tc.tile_wait_until(0.05):
                nc.sync.dma_start(
                    th[:, :].rearrange("p (t i) -> p t i", i=16),
                    t16_sb[:, 2048 * c:2048 * (c + 1)],
                    transpose=True)
            cs = work.tile([128, 128], F32, name="cs", tag="cs")
            cs4 = cs[:, :].rearrange("p (t w x) -> p t w x", w=4, x=2)
            th3 = th[:, :].rearrange("p (t i) -> p t i", i=16)[:, :, 0:4]
            # cos(theta) = sin(pi/4*tanh + 3pi/4); sin(theta) = sin(.. + pi/4)
            nc.scalar.activation(cs4[:, :, :, 0], th3, AF.Sin,
                                 bias=trigb_sb[:, 0:1], scale=PI4)
            nc.scalar.activation(cs4[:, :, :, 1], th3, AF.Sin,
                                 bias=trigb_sb[:, 1:2], scale=PI4)
            cs8 = cs[:, :].rearrange("p (t w) -> p t w", w=8)
            v01 = work.tile([128, 64], F32, name="v01", tag="v01")
            v23 = work.tile([128, 64], F32, name="v23", tag="v23")
            nc.vector.tensor_tensor(
                out=v01[:, :].rearrange("p (t a b) -> p t a b", a=2, b=2),
                in0=cs8[:, :, 0:2].unsqueeze(3).broadcast_to((128, 16, 2, 2)),
                in1=cs8[:, :, 2:4].unsqueeze(2).broadcast_to((128, 16, 2, 2)),
                op=mybir.AluOpType.mult)
            nc.vector.tensor_tensor(
                out=v23[:, :].rearrange("p (t a b) -> p t a b", a=2, b=2),
                in0=cs8[:, :, 4:6].unsqueeze(3).broadcast_to((128, 16, 2, 2)),
                in1=cs8[:, :, 6:8].unsqueeze(2).broadcast_to((128, 16, 2, 2)),
                op=mybir.AluOpType.mult)
            ps = work.tile([128, 256], F32, name="psi", tag="psi")
            nc.vector.tensor_tensor(
                out=ps[:, :].rearrange("p (t a b) -> p t a b", a=4, b=4),
                in0=v01[:, :].rearrange("p (t i) -> p t i", i=4)
                    .unsqueeze(3).broadcast_to((128, 16, 4, 4)),
                in1=v23[:, :].rearrange("p (t i) -> p t i", i=4)
                    .unsqueeze(2).broadcast_to((128, 16, 4, 4)),
                op=mybir.AluOpType.mult)
            psi[c] = ps

        def emit_transpose(c):
            """PE transpose of both 128-col halves of psi[c] into one
            [128, 256] PSUM tile + single DVE copy to bf16 SBUF."""
            psiT_ps = ps2.tile([128, 256], F32, name="psiT_ps", tag="p2")
            for h in range(2):
                nc.tensor.transpose(
                    psiT_ps[:, 128 * h:128 * (h + 1)],
                    psi[c][:, 128 * h:128 * (h + 1)], id_sb)
            pt = work.tile([128, 256], BF16, name="psiT", tag="psiT")
            nc.vector.tensor_copy(pt, psiT_ps)  # f32 psum -> bf16 sbuf
            psiT[c] = pt

        def emit_msqp(c):
            """Block-diagonal M matmul, square, P matmul, +post_b for the
            whole chunk c (N=256 moving: both halves in one instruction)."""
            phiT_ps = ps2.tile([128, 256], F32, name="phiT_ps", tag="p2")
            nc.tensor.matmul(phiT_ps, lhsT=mbd_sb, rhs=psiT[c],
                             start=True, stop=True)
            ph = work.tile([128, 256], BF16, name="phi2", tag="phi2")
            # DVE cannot read two non-scalar PSUM inputs -> Square on ScalarE
            nc.scalar.activation(ph, phiT_ps, AF.Square)
            phi2[c] = ph
            o10_ps = ps2.tile([80, 256], F32, name="o10_ps", tag="p2")
            nc.tensor.matmul(o10_ps, lhsT=pbd_sb, rhs=ph,
                             start=True, stop=True)
            # +post_b on the DVE, landing in the transposed output staging
            nc.vector.tensor_tensor(
                out=out2_sb[:, 256 * c:256 * (c + 1)],
                in0=o10_ps, in1=pb80_sb.broadcast_to((80, 256)),
                op=mybir.AluOpType.add)
            o10[c] = o10_ps

        # software-pipelined emission: for chunk c, trig follows unit 2c+1,
        # the PE transposes lag one chunk and M/sq/P two chunks, so no
        # engine FIFO head-blocks on a cross-engine dep that is not ready
        for u in range(UNITS):
            emit_pre(u)
            if u < UNITS - 1:
                emit_fill(5)
            if u % 2 == 1:
                c = u // 2
                emit_trig(c)
                if c >= 1:
                    emit_transpose(c - 1)
                if c >= 2:
                    emit_msqp(c - 2)
        emit_transpose(3)
        emit_msqp(2)
        emit_msqp(3)

        # single store at the very end, in transposed layout [64, 10, 128];
        # the host flips it back to [8192, 10].  SP ring; sim-time hint
        # keeps its semaphore lane behind the transposes.
        with tc.tile_wait_until(0.06):
            nc.sync.dma_start(
                out[:, :, :].rearrange("(h t) c p -> (t c) h p", h=UNITS),
                out2_sb[:, :].rearrange("p (h b) -> p h b", h=UNITS))

    nc.finalize()  # bacc: register alloc + event-semaphore wait splitting
    return nc


_NC_CACHE: dict = {}


def _get_nc() -> bass.Bass:
    if "nc" not in _NC_CACHE:
        _NC_CACHE["nc"] = build_nc()
    return _NC_CACHE["nc"]


def make_in_maps(inputs: dict) -> list:
    x = np.asarray(inputs["input_features"], np.float32)
    pre_w = np.asarray(inputs["pre_w"], np.float32)
    pre_b = np.asarray(inputs["pre_b"], np.float32)
    q_params = np.asarray(inputs["q_params"], np.float32)
    post_w = np.asarray(inputs["post_w"], np.float32)
    post_b = np.asarray(inputs["post_b"], np.float32)

    M = _build_M(q_params)
    P = _build_P(post_w)
    cbf = np.zeros((128, CBF_COLS), np.float32)
    # pre_wt[p, 4k+f] = pre_w[f, 128k+p]
    cbf[:, PW0:PW0 + 16] = (
        pre_w.T.reshape(4, 128, 4).transpose(1, 0, 2).reshape(128, 16))
    for t in range(8):
        cbf[16 * t:16 * (t + 1), MB0 + 16 * t:MB0 + 16 * (t + 1)] = M.T
        cbf[16 * t:16 * (t + 1), PB0 + 10 * t:PB0 + 10 * (t + 1)] = P
    cbf = cbf.astype(ml_dtypes.bfloat16)

    cf32 = np.zeros((128, CF32_COLS), np.float32)
    cf32[0:4, 0] = pre_b
    cf32[0:80, 1] = np.tile(post_b, 8)
    cf32[:, 2] = 3.0 * PI4
    cf32[:, 3] = PI4
    cf32[:, 4:132] = np.eye(128)
    xb = x.astype(ml_dtypes.bfloat16)
    consts = dict(cbf=cbf, cf32=cf32)
    in_maps = []
    for i in range(N_CORES):
        # xt[u][p, 1024k+b] = x_core[1024u+b, 128k+p]
        xc = xb[B * i:B * (i + 1)]                       # [8192, 512]
        xct = np.ascontiguousarray(
            xc.reshape(UNITS, UB, 4, 128).transpose(0, 3, 2, 1)
        ).reshape(UNITS, 128, 4 * UB)
        in_maps.append(dict(
            x0=xct[0], x1=xct[1],
            x23=np.concatenate([xct[2], xct[3]], axis=1),
            x45=np.concatenate([xct[4], xct[5]], axis=1),
            x67=np.concatenate([xct[6], xct[7]], axis=1),
            **consts))
    return in_maps


def unpack_out(dev_out: np.ndarray) -> np.ndarray:
    """[TILES, C, 128] device layout -> [B, C]."""
    return dev_out.transpose(0, 2, 1).reshape(B, C)


def run_on_device(inputs: dict, **kwargs):
    """Returns (full_output, BassKernelResults)."""
    nc = _get_nc()
    in_maps = make_in_maps(inputs)
    res = run_bass_kernel_spmd(nc, in_maps, core_ids=list(range(N_CORES)),
                               **kwargs)
    full = np.concatenate(
        [unpack_out(res.results[i]["out"]) for i in range(N_CORES)], 0)
    return np.ascontiguousarray(full, dtype=np.float32), res


def kernel(**inputs) -> np.ndarray:
    out, _ = run_on_device(inputs)
    return out


# revision 13
# speedup vs baseline: 1.0008x; 1.0008x over previous
"""Trainium2 Bass kernel for nn_DressedQuantumNet.

Math reformulation (exact, up to float rounding):
  pre_out = x @ pre_w.T + pre_b                  # [B,4]
  theta_w = (pi/4)*tanh(pre_out_w) + pi/4        # in (0, pi/2)
  v_w     = [cos theta_w, sin theta_w]           # per-qubit state (positive)
  psi     = v_0 (x) v_1 (x) v_2 (x) v_3          # [B,16] product state
  phi     = M @ psi        # M = fixed 16x16 matrix of the CNOT/RY circuit
  out     = (phi*phi)^T P + post_b  # P[i,c] = sum_w post_w[c,w] * z_w(i)

Device strategy (pure data parallel over 8 cores, 8192 samples each):
  - x is downcast to bf16 AND transposed on host into per-unit slabs
    xt[u, p, 1024k+b] = x[1024u+b, 128k+p]. The device then only does
    plain linear DMA at full HBM bandwidth (~358 GB/s); the on-device
    xbar-transpose path measured ~216 GB/s and ran on the critical path.
  - 8 units of 1024 samples pipeline through: pre-matmul (tiny 4-column
    LDWEIGHTS, x streaming 512 cols/instr, PSUM [4,1024] over 2 banks),
    fused bias+tanh (ScalarE, bf16 out), one small SBUF->SBUF xbar
    transpose back to sample-major, 2x Sin (cos folded via bias), psi
    via 3 broadcast-AP DVE multiplies, PE transpose of psi, then two
    block-diagonal bf16 matmuls (M and P).
  - phase-2 weights/data in bf16 (host-sim rel err 0.0086 vs 0.0066
    all-f32; gate is 2e-2) -- bf16 matmuls stream 2x faster than f32
    and FWL halves their LDWEIGHTS cost.
  - queue split: SP ring = const + x slab triggers (plain copies only);
    ACT ring = the 8 xbar transposes + final store; Square and the
    +post_b bias run on the otherwise-idle DVE so ScalarE only does
    tanh/sin. Phase-2 PE work for unit u is emitted after unit u+3's
    matmuls so no engine FIFO ever head-blocks on a cross-engine dep.
  - rows 4..15 of the tanh staging tile are zero-filled once by DMA from
    a DRAM zeros buffer (junk would be correct -- never read back -- but
    CoreSim flags uninitialized reads).
"""

import os
import sys

for _p in ("/opt/trn_rl_repo",):
    if os.path.isdir(_p) and _p not in sys.path:
        sys.path.insert(0, _p)

import math
import numpy as np
import ml_dtypes
from contextlib import ExitStack

import concourse.bass as bass
import concourse.bacc as bacc
import concourse.mybir as mybir
from concourse.tile import TileContext
from concourse.bass_utils import run_bass_kernel_spmd

F32 = mybir.dt.float32
BF16 = mybir.dt.bfloat16
AF = mybir.ActivationFunctionType
PI4 = math.pi / 4.0

N_CORES = 8
B_FULL, D, C = 65536, 512, 10
B = B_FULL // N_CORES          # 8192 samples per core
N_QUBITS, Q_DEPTH = 4, 6
TILES = B // 128               # 64 sample tiles of 128
UNITS = 8                      # pipeline units of 1024 samples
UB = B // UNITS                # 1024 samples per unit

# packed bf16 const columns: pre_wt | mbd | pbd
PW0, MB0, PB0 = 0, 16, 144
CBF_COLS = 224
# packed f32 const columns: pre_b | post_b | trigb x2 | ident (f32: the
# psi transpose operands must both be f32)
CF32_COLS = 132


# ---------------------------------------------------------------- host math
def _apply_1q(state, gate, wire):
    state = np.moveaxis(state, wire, 0)
    state = np.tensordot(gate, state, axes=((1,), (0,)))
    return np.moveaxis(state, 0, wire)


def _apply_cnot(state, ctrl, tgt):
    state = np.moveaxis(state, (ctrl, tgt), (0, 1))
    state = np.stack([state[0], state[1][::-1]], axis=0)
    return np.moveaxis(state, (0, 1), (ctrl, tgt))


def _ry(theta):
    c, s = np.cos(theta * 0.5), np.sin(theta * 0.5)
    return np.array([[c, -s], [s, c]])


def _build_M(q_params: np.ndarray) -> np.ndarray:
    """16x16 matrix of the fixed part of the circuit (after the per-sample
    RY layer): 6 repetitions of [CNOT(0,1), CNOT(2,3), CNOT(1,2), RY layer]."""
    qw = np.asarray(q_params, np.float64).reshape(Q_DEPTH, N_QUBITS)
    M = np.zeros((16, 16), np.float64)
    for i in range(16):
        state = np.zeros(16, np.float64)
        state[i] = 1.0
        state = state.reshape((2,) * N_QUBITS)
        for k in range(Q_DEPTH):
            for a in range(0, N_QUBITS - 1, 2):
                state = _apply_cnot(state, a, a + 1)
            for a in range(1, N_QUBITS - 1, 2):
                state = _apply_cnot(state, a, a + 1)
            for w in range(N_QUBITS):
                state = _apply_1q(state, _ry(qw[k, w]), w)
        M[:, i] = state.reshape(16)
    return M


def _build_P(post_w: np.ndarray) -> np.ndarray:
    """P[i, c] = sum_w post_w[c, w] * z_w(i), where z_w(i) flips sign with
    bit (3-w) of the state index i (axis 0 of the state = qubit 0)."""
    post_w = np.asarray(post_w, np.float64)
    i = np.arange(16)
    z = np.stack([1.0 - 2.0 * ((i >> (3 - w)) & 1) for w in range(N_QUBITS)], 1)
    return z @ post_w.T  # [16, 10]


# ---------------------------------------------------------------- bass build
def build_nc(sim_compat: bool = False) -> bass.Bass:
    # Bacc (not raw Bass): its finalize() runs generate_event_semaphores,
    # which splits multi-semaphore waits to satisfy the TRN2 one-wait-per-
    # instruction ISA limit.
    nc = bacc.Bacc(None)
    # xt[u][p, 1024k+b] = x[1024u+b, 128k+p]: host-pretransposed bf16.
    # Five slab tensors (1MB, 1MB, 2MB, 2MB, 2MB): HWDGE DMAs share 8
    # completion semaphores round-robin in SCHEDULED order and DMA #N
    # waits on #N-8, so all 7 load DMAs must be scheduled before the
    # late-running xbar transposes (see the tile_wait_until hints below)
    # or slab loads serialize behind them.
    x0 = nc.dram_tensor("x0", [128, 4 * UB], BF16, kind="ExternalInput")
    x1 = nc.dram_tensor("x1", [128, 4 * UB], BF16, kind="ExternalInput")
    x23 = nc.dram_tensor("x23", [128, 8 * UB], BF16, kind="ExternalInput")
    x45 = nc.dram_tensor("x45", [128, 8 * UB], BF16, kind="ExternalInput")
    x67 = nc.dram_tensor("x67", [128, 8 * UB], BF16, kind="ExternalInput")
    cbf = nc.dram_tensor("cbf", [128, CBF_COLS], BF16, kind="ExternalInput")
    cf32 = nc.dram_tensor("cf32", [128, CF32_COLS], F32, kind="ExternalInput")
    # transposed on device: out[tile, class, sample-in-tile]; host flips back
    out = nc.dram_tensor("out", [TILES, C, 128], F32, kind="ExternalOutput")

    with ExitStack() as ctx:
        tc = ctx.enter_context(TileContext(nc))
        consts = ctx.enter_context(tc.tile_pool(name="consts", bufs=1))
        # all 8 x slabs stay resident (8.4 MB): every load is issued up
        # front on the SP ring and compute streams through in order
        xt_pool = ctx.enter_context(tc.tile_pool(name="xt", bufs=5))
        work = ctx.enter_context(tc.tile_pool(name="work", bufs=3))
        ps_po = ctx.enter_context(tc.tile_pool(name="ps_po", space="PSUM", bufs=2))
        ps2 = ctx.enter_context(tc.tile_pool(name="ps2", space="PSUM", bufs=5))
        fill_ps = ctx.enter_context(tc.tile_pool(name="fill", space="PSUM", bufs=1))

        # consts first on the SP ring; cf32 first so the activation-table
        # pin (which reads it) runs before the first tanh needs the table
        cf32_sb = consts.tile([128, CF32_COLS], F32)
        nc.sync.dma_start(cf32_sb, cf32[:, :])
        cbf_sb = consts.tile([128, CBF_COLS], BF16)
        nc.sync.dma_start(cbf_sb, cbf[:, :])
        pre_wt_sb = cbf_sb[:, PW0:PW0 + 16]
        mbd_sb = cbf_sb[:, MB0:MB0 + 128]
        pbd_sb = cbf_sb[:, PB0:PB0 + 80]
        pre_b_sb = cf32_sb[0:4, 0:1]
        pb80_sb = cf32_sb[0:80, 1:2]
        trigb_sb = cf32_sb[:, 2:4]
        id_sb = cf32_sb[:, 4:132]

        out2_sb = consts.tile([80, 128 * UNITS], F32)  # [80, 1024]
        # feature-row tanh staging, bf16, padded to 16 partitions so the
        # SBUF->SBUF xbar transpose (rows%16==0) can flip it to sample-major
        t16_sb = consts.tile([16, B], BF16)  # [16, 8192]
        # zero it on the otherwise-idle GpSimd engine in per-chunk pieces so
        # chunk 0's transpose is unblocked early (junk in rows 4..15 would be
        # correct -- never read back -- but CoreSim flags uninit reads)
        for c in range(4):
            nc.gpsimd.memset(t16_sb[:, 2048 * c:2048 * (c + 1)], 0.0)

        # pin the activation table to silu_and_others once: it contains
        # silu+tanh+sin+square+identity, so no further table loads happen.
        # (CoreSim can't evaluate Silu; the sim build substitutes Tanh --
        # the value is unused either way.)
        silu_sb = consts.tile([128, 1], F32)
        nc.scalar.activation(silu_sb, cf32_sb[:, 2:3],
                             AF.Tanh if sim_compat else AF.Silu)

        # filler matmuls: junk matmuls on resident const data that keep
        # the PE's activity monitor busy during DMA gaps. The HAM clock
        # gate halves the PE clock (1.2 vs 2.4 GHz) unless the PE has been
        # continuously busy for ~3.4us -- with the kernel DMA-paced the PE
        # has ~40% idle and every real matmul ran at the cold rate.
        fill_sb = fill_ps.tile([4, 512], F32)

        def emit_fill(n, ncols=224):
            for _ in range(n):
                nc.tensor.matmul(fill_sb[:, 0:ncols], lhsT=cbf_sb[:, 0:4],
                                 rhs=cbf_sb[:, 0:ncols], start=True, stop=True)

        # issue ALL x slab loads now, in consumption order, on the SP ring
        xt_sbs = []  # per unit: (tile, col offset)
        for dram, n_units in ((x0, 1), (x1, 1), (x23, 2), (x45, 2), (x67, 2)):
            xt_sb = xt_pool.tile([128, n_units * 4 * UB], BF16,
                                 name="xt", tag="xt")
            nc.sync.dma_start(xt_sb, dram[:, :])
            for j in range(n_units):
                xt_sbs.append((xt_sb, j * 4 * UB))

        # pre-warm the PE while the first x slab is still in flight
        emit_fill(18, 128)

        # per-unit tiles kept for cross-stage use (unit = 1024 samples)
        psi = [None] * UNITS
        psiT = [None] * UNITS
        phi2 = [None] * UNITS

        def emit_pre(u):
            """pre-net matmuls + fused bias/tanh for unit u (2 half-units
            of 512 samples: single-bank PSUM tiles, shorter tanh)."""
            xt_sb, off = xt_sbs[u]
            for j in range(2):
                po = ps_po.tile([4, 512], F32, name="po", tag="po")
                for k in range(4):
                    c0 = off + UB * k + 512 * j
                    nc.tensor.matmul(
                        po[:, :],
                        lhsT=pre_wt_sb[:, 4 * k:4 * (k + 1)],
                        rhs=xt_sb[:, c0:c0 + 512],
                        start=(k == 0), stop=(k == 3))
                nc.scalar.activation(
                    t16_sb[0:4, UB * u + 512 * j:UB * u + 512 * (j + 1)],
                    po, AF.Tanh, bias=pre_b_sb)

        def emit_trig(u):
            """xbar flip to sample-major + trig + psi products for unit u."""
            # th_bf[p, t, i] = t16[i, 1024u + 128t + p].  SP-ring xbar; the
            # tile_wait_until hint is SIM-TIME ONLY: it pushes this DMA late
            # in the scheduled order so the 8 round-robin HWDGE semaphore
            # lanes see [7 loads, then transposes] and no load ever waits
            # on a late transpose. Real execution follows data deps only.
            th = work.tile([128, 128], BF16, name="th_bf", tag="th_bf")
            with tc.tile_wait_until(0.05):
                nc.sync.dma_start(
                    th[:, :].rearrange("p (t i) -> p t i", i=16),
                    t16_sb[:, UB * u:UB * (u + 1)],
                    transpose=True)
            cs = work.tile([128, 64], F32, name="cs", tag="cs")
            cs4 = cs[:, :].rearrange("p (t w x) -> p t w x", w=4, x=2)
            th3 = th[:, :].rearrange("p (t i) -> p t i", i=16)[:, :, 0:4]
            # cos(theta) = sin(pi/4*tanh + 3pi/4); sin(theta) = sin(.. + pi/4)
            nc.scalar.activation(cs4[:, :, :, 0], th3, AF.Sin,
                                 bias=trigb_sb[:, 0:1], scale=PI4)
            nc.scalar.activation(cs4[:, :, :, 1], th3, AF.Sin,
                                 bias=trigb_sb[:, 1:2], scale=PI4)
            cs8 = cs[:, :].rearrange("p (t w) -> p t w", w=8)
            v01 = work.tile([128, 32], F32, name="v01", tag="v01")
            v23 = work.tile([128, 32], F32, name="v23", tag="v23")
            nc.vector.tensor_tensor(
                out=v01[:, :].rearrange("p (t a b) -> p t a b", a=2, b=2),
                in0=cs8[:, :, 0:2].unsqueeze(3).broadcast_to((128, 8, 2, 2)),
                in1=cs8[:, :, 2:4].unsqueeze(2).broadcast_to((128, 8, 2, 2)),
                op=mybir.AluOpType.mult)
            nc.vector.tensor_tensor(
                out=v23[:, :].rearrange("p (t a b) -> p t a b", a=2, b=2),
                in0=cs8[:, :, 4:6].unsqueeze(3).broadcast_to((128, 8, 2, 2)),
                in1=cs8[:, :, 6:8].unsqueeze(2).broadcast_to((128, 8, 2, 2)),
                op=mybir.AluOpType.mult)
            ps = work.tile([128, 128], F32, name="psi", tag="psi")
            nc.vector.tensor_tensor(
                out=ps[:, :].rearrange("p (t a b) -> p t a b", a=4, b=4),
                in0=v01[:, :].rearrange("p (t i) -> p t i", i=4)
                    .unsqueeze(3).broadcast_to((128, 8, 4, 4)),
                in1=v23[:, :].rearrange("p (t i) -> p t i", i=4)
                    .unsqueeze(2).broadcast_to((128, 8, 4, 4)),
                op=mybir.AluOpType.mult)
            psi[u] = ps

        def emit_quantum(u):
            """PE transpose, M matmul, square, P matmul, +post_b, store
            for unit u."""
            psiT_ps = ps2.tile([128, 128], F32, name="psiT_ps", tag="p2")
            nc.tensor.transpose(psiT_ps, psi[u][:, :], id_sb)
            pt = work.tile([128, 128], BF16, name="psiT", tag="psiT")
            nc.vector.tensor_copy(pt, psiT_ps)  # f32 psum -> bf16 sbuf
            psiT[u] = pt
            phiT_ps = ps2.tile([128, 128], F32, name="phiT_ps", tag="p2")
            nc.tensor.matmul(phiT_ps, lhsT=mbd_sb, rhs=pt,
                             start=True, stop=True)
            ph = work.tile([128, 128], BF16, name="phi2", tag="phi2")
            # DVE cannot read two non-scalar PSUM inputs -> Square on ScalarE
            nc.scalar.activation(ph, phiT_ps, AF.Square)
            phi2[u] = ph
            o10_ps = ps2.tile([80, 128], F32, name="o10_ps", tag="p2")
            nc.tensor.matmul(o10_ps, lhsT=pbd_sb, rhs=ph,
                             start=True, stop=True)
            # +post_b on the DVE, landing in the transposed output staging
            nc.vector.tensor_tensor(
                out=out2_sb[:, 128 * u:128 * (u + 1)],
                in0=o10_ps, in1=pb80_sb.broadcast_to((80, 128)),
                op=mybir.AluOpType.add)
            # store this unit's [8 tiles, 10 classes, 128 samples] now; the
            # sim-time hint keeps its semaphore lane behind the transposes
            with tc.tile_wait_until(0.06):
                nc.sync.dma_start(
                    out[8 * u:8 * (u + 1), :, :]
                        .rearrange("t c p -> (t c) p"),
                    out2_sb[:, 128 * u:128 * (u + 1)])

        # software-pipelined emission: trig lags pre by 1 unit, quantum by
        # 2, so no engine FIFO head-blocks on a cross-engine dep that is
        # not ready; fillers keep the PE's activity monitor hot in the gaps
        for u in range(UNITS):
            emit_pre(u)
            if u < UNITS - 1:
                emit_fill(4)
            if u >= 1:
                emit_trig(u - 1)
            if u >= 2:
                emit_quantum(u - 2)
        emit_trig(UNITS - 1)
        emit_quantum(UNITS - 2)
        emit_quantum(UNITS - 1)

    nc.finalize()  # bacc: register alloc + event-semaphore wait splitting
    return nc


_NC_CACHE: dict = {}


def _get_nc() -> bass.Bass:
    if "nc" not in _NC_CACHE:
        _NC_CACHE["nc"] = build_nc()
    return _NC_CACHE["nc"]


def make_in_maps(inputs: dict) -> list:
    x = np.asarray(inputs["input_features"], np.float32)
    pre_w = np.asarray(inputs["pre_w"], np.float32)
    pre_b = np.asarray(inputs["pre_b"], np.float32)
    q_params = np.asarray(inputs["q_params"], np.float32)
    post_w = np.asarray(inputs["post_w"], np.float32)
    post_b = np.asarray(inputs["post_b"], np.float32)

    M = _build_M(q_params)
    P = _build_P(post_w)
    cbf = np.zeros((128, CBF_COLS), np.float32)
    # pre_wt[p, 4k+f] = pre_w[f, 128k+p]
    cbf[:, PW0:PW0 + 16] = (
        pre_w.T.reshape(4, 128, 4).transpose(1, 0, 2).reshape(128, 16))
    for t in range(8):
        cbf[16 * t:16 * (t + 1), MB0 + 16 * t:MB0 + 16 * (t + 1)] = M.T
        cbf[16 * t:16 * (t + 1), PB0 + 10 * t:PB0 + 10 * (t + 1)] = P
    cbf = cbf.astype(ml_dtypes.bfloat16)

    cf32 = np.zeros((128, CF32_COLS), np.float32)
    cf32[0:4, 0] = pre_b
    cf32[0:80, 1] = np.tile(post_b, 8)
    cf32[:, 2] = 3.0 * PI4
    cf32[:, 3] = PI4
    cf32[:, 4:132] = np.eye(128)
    xb = x.astype(ml_dtypes.bfloat16)
    consts = dict(cbf=cbf, cf32=cf32)
    in_maps = []
    for i in range(N_CORES):
        # xt[u][p, 1024k+b] = x_core[1024u+b, 128k+p]
        xc = xb[B * i:B * (i + 1)]                       # [8192, 512]
        xct = np.ascontiguousarray(
            xc.reshape(UNITS, UB, 4, 128).transpose(0, 3, 2, 1)
        ).reshape(UNITS, 128, 4 * UB)
        in_maps.append(dict(
            x0=xct[0], x1=xct[1],
            x23=np.concatenate([xct[2], xct[3]], axis=1),
            x45=np.concatenate([xct[4], xct[5]], axis=1),
            x67=np.concatenate([xct[6], xct[7]], axis=1),
            **consts))
    return in_maps


def unpack_out(dev_out: np.ndarray) -> np.ndarray:
    """[TILES, C, 128] device layout -> [B, C]."""
    return dev_out.transpose(0, 2, 1).reshape(B, C)


def run_on_device(inputs: dict, **kwargs):
    """Returns (full_output, BassKernelResults)."""
    nc = _get_nc()
    in_maps = make_in_maps(inputs)
    res = run_bass_kernel_spmd(nc, in_maps, core_ids=list(range(N_CORES)),
                               **kwargs)
    full = np.concatenate(
        [unpack_out(res.results[i]["out"]) for i in range(N_CORES)], 0)
    return np.ascontiguousarray(full, dtype=np.float32), res


def kernel(**inputs) -> np.ndarray:
    out, _ = run_on_device(inputs)
    return out


# revision 15
# speedup vs baseline: 1.1093x; 1.1084x over previous
"""Trainium2 Bass kernel for nn_DressedQuantumNet.

Math reformulation (exact, up to float rounding):
  pre_out = x @ pre_w.T + pre_b                  # [B,4]
  theta_w = (pi/4)*tanh(pre_out_w) + pi/4        # in (0, pi/2)
  v_w     = [cos theta_w, sin theta_w]           # per-qubit state (positive)
  psi     = v_0 (x) v_1 (x) v_2 (x) v_3          # [B,16] product state
  phi     = M @ psi        # M = fixed 16x16 matrix of the CNOT/RY circuit
  out     = (phi*phi)^T P + post_b  # P[i,c] = sum_w post_w[c,w] * z_w(i)

Device strategy (pure data parallel over 8 cores, 8192 samples each):
  - x is downcast to bf16 AND transposed on host into per-unit slabs
    xt[u, p, 1024k+b] = x[1024u+b, 128k+p]. The device then only does
    plain linear DMA at full HBM bandwidth (~358 GB/s); the on-device
    xbar-transpose path measured ~216 GB/s and ran on the critical path.
  - 8 units of 1024 samples pipeline through: pre-matmul (tiny 4-column
    LDWEIGHTS, x streaming 512 cols/instr, PSUM [4,1024] over 2 banks),
    fused bias+tanh (ScalarE, bf16 out), one small SBUF->SBUF xbar
    transpose back to sample-major, 2x Sin (cos folded via bias), psi
    via 3 broadcast-AP DVE multiplies, PE transpose of psi, then two
    block-diagonal bf16 matmuls (M and P).
  - phase-2 weights/data in bf16 (host-sim rel err 0.0086 vs 0.0066
    all-f32; gate is 2e-2) -- bf16 matmuls stream 2x faster than f32
    and FWL halves their LDWEIGHTS cost.
  - queue split: SP ring = const + x slab triggers (plain copies only);
    ACT ring = the 8 xbar transposes + final store; Square and the
    +post_b bias run on the otherwise-idle DVE so ScalarE only does
    tanh/sin. Phase-2 PE work for unit u is emitted after unit u+3's
    matmuls so no engine FIFO ever head-blocks on a cross-engine dep.
  - rows 4..15 of the tanh staging tile are zero-filled once by DMA from
    a DRAM zeros buffer (junk would be correct -- never read back -- but
    CoreSim flags uninitialized reads).
"""

import os
import sys

for _p in ("/opt/trn_rl_repo",):
    if os.path.isdir(_p) and _p not in sys.path:
        sys.path.insert(0, _p)

import math
import numpy as np
import ml_dtypes
from contextlib import ExitStack

import concourse.bass as bass
import concourse.bacc as bacc
import concourse.mybir as mybir
from concourse.tile import TileContext
from concourse.bass_utils import run_bass_kernel_spmd

F32 = mybir.dt.float32
BF16 = mybir.dt.bfloat16
AF = mybir.ActivationFunctionType
PI4 = math.pi / 4.0

N_CORES = 8
B_FULL, D, C = 65536, 512, 10
B = B_FULL // N_CORES          # 8192 samples per core
N_QUBITS, Q_DEPTH = 4, 6
TILES = B // 128               # 64 sample tiles of 128
UNITS = 8                      # pipeline units of 1024 samples
UB = B // UNITS                # 1024 samples per unit

# packed bf16 const columns: pre_wt | mbd | pbd
PW0, MB0, PB0 = 0, 16, 144
CBF_COLS = 224
# packed f32 const columns: post_b | trigb x2 | ident | pre_b32 (f32: the
# psi transpose operands must both be f32)
CF32_COLS = 163


# ---------------------------------------------------------------- host math
def _apply_1q(state, gate, wire):
    state = np.moveaxis(state, wire, 0)
    state = np.tensordot(gate, state, axes=((1,), (0,)))
    return np.moveaxis(state, 0, wire)


def _apply_cnot(state, ctrl, tgt):
    state = np.moveaxis(state, (ctrl, tgt), (0, 1))
    state = np.stack([state[0], state[1][::-1]], axis=0)
    return np.moveaxis(state, (0, 1), (ctrl, tgt))


def _ry(theta):
    c, s = np.cos(theta * 0.5), np.sin(theta * 0.5)
    return np.array([[c, -s], [s, c]])


def _build_M(q_params: np.ndarray) -> np.ndarray:
    """16x16 matrix of the fixed part of the circuit (after the per-sample
    RY layer): 6 repetitions of [CNOT(0,1), CNOT(2,3), CNOT(1,2), RY layer]."""
    qw = np.asarray(q_params, np.float64).reshape(Q_DEPTH, N_QUBITS)
    M = np.zeros((16, 16), np.float64)
    for i in range(16):
        state = np.zeros(16, np.float64)
        state[i] = 1.0
        state = state.reshape((2,) * N_QUBITS)
        for k in range(Q_DEPTH):
            for a in range(0, N_QUBITS - 1, 2):
                state = _apply_cnot(state, a, a + 1)
            for a in range(1, N_QUBITS - 1, 2):
                state = _apply_cnot(state, a, a + 1)
            for w in range(N_QUBITS):
                state = _apply_1q(state, _ry(qw[k, w]), w)
        M[:, i] = state.reshape(16)
    return M


def _build_P(post_w: np.ndarray) -> np.ndarray:
    """P[i, c] = sum_w post_w[c, w] * z_w(i), where z_w(i) flips sign with
    bit (3-w) of the state index i (axis 0 of the state = qubit 0)."""
    post_w = np.asarray(post_w, np.float64)
    i = np.arange(16)
    z = np.stack([1.0 - 2.0 * ((i >> (3 - w)) & 1) for w in range(N_QUBITS)], 1)
    return z @ post_w.T  # [16, 10]


# ---------------------------------------------------------------- bass build
def build_nc(sim_compat: bool = False) -> bass.Bass:
    # Bacc (not raw Bass): its finalize() runs generate_event_semaphores,
    # which splits multi-semaphore waits to satisfy the TRN2 one-wait-per-
    # instruction ISA limit.
    nc = bacc.Bacc(None)
    # xt[u][p, 1024k + 128t + b] = x[1024u + 128t + b, 128k + p]: host-
    # pretransposed bf16, grouped so each 128-sample tile's feature chunk
    # is one contiguous 128x128 LDWEIGHTS block.  Six slab tensors; with
    # the 2 const loads that is exactly 8 early HWDGE DMAs -- the 8
    # round-robin completion-semaphore lanes mean DMA #N waits on #N-8,
    # so every late DMA (the stores) then waits only on an early load.
    x0 = nc.dram_tensor("x0", [128, 4 * UB], BF16, kind="ExternalInput")
    x1 = nc.dram_tensor("x1", [128, 4 * UB], BF16, kind="ExternalInput")
    x23 = nc.dram_tensor("x23", [128, 8 * UB], BF16, kind="ExternalInput")
    x45 = nc.dram_tensor("x45", [128, 8 * UB], BF16, kind="ExternalInput")
    x6 = nc.dram_tensor("x6", [128, 4 * UB], BF16, kind="ExternalInput")
    x7 = nc.dram_tensor("x7", [128, 4 * UB], BF16, kind="ExternalInput")
    cbf = nc.dram_tensor("cbf", [128, CBF_COLS], BF16, kind="ExternalInput")
    cf32 = nc.dram_tensor("cf32", [128, CF32_COLS], F32, kind="ExternalInput")
    # transposed on device: out[tile, class, sample-in-tile]; host flips back
    out = nc.dram_tensor("out", [TILES, C, 128], F32, kind="ExternalOutput")

    with ExitStack() as ctx:
        tc = ctx.enter_context(TileContext(nc))
        consts = ctx.enter_context(tc.tile_pool(name="consts", bufs=1))
        # all 8 x slabs stay resident (8.4 MB): every load is issued up
        # front on the SP ring and compute streams through in order
        xt_pool = ctx.enter_context(tc.tile_pool(name="xt", bufs=6))
        work = ctx.enter_context(tc.tile_pool(name="work", bufs=3))
        ps_po = ctx.enter_context(tc.tile_pool(name="ps_po", space="PSUM", bufs=2))
        ps2 = ctx.enter_context(tc.tile_pool(name="ps2", space="PSUM", bufs=5))
        fill_ps = ctx.enter_context(tc.tile_pool(name="fill", space="PSUM", bufs=1))

        cf32_sb = consts.tile([128, CF32_COLS], F32)
        nc.sync.dma_start(cf32_sb, cf32[:, :])
        cbf_sb = consts.tile([128, CBF_COLS], BF16)
        nc.sync.dma_start(cbf_sb, cbf[:, :])
        pre_wt_sb = cbf_sb[:, PW0:PW0 + 16]
        mbd_sb = cbf_sb[:, MB0:MB0 + 128]
        pbd_sb = cbf_sb[:, PB0:PB0 + 80]
        pb80_sb = cf32_sb[0:80, 0:1]
        trigb_sb = cf32_sb[:, 1:3]
        id_sb = cf32_sb[:, 3:131]
        preb32_sb = cf32_sb[:, 131:163]

        out2_sb = consts.tile([80, 128 * UNITS], F32)  # [80, 1024]

        # pin the activation table to silu_and_others once: it contains
        # silu+tanh+sin+square+identity, so no further table loads happen.
        # (CoreSim can't evaluate Silu; the sim build substitutes Tanh --
        # the value is unused either way.)
        silu_sb = consts.tile([128, 1], F32)
        nc.scalar.activation(silu_sb, cf32_sb[:, 1:2],
                             AF.Tanh if sim_compat else AF.Silu)

        # filler matmuls: junk matmuls on resident const data that keep
        # the PE's activity monitor busy during DMA gaps. The HAM clock
        # gate halves the PE clock (1.2 vs 2.4 GHz) unless the PE has been
        # continuously busy for ~3.4us.
        fill_sb = fill_ps.tile([4, 512], F32)

        def emit_fill(n, ncols=224):
            for _ in range(n):
                nc.tensor.matmul(fill_sb[:, 0:ncols], lhsT=cbf_sb[:, 0:4],
                                 rhs=cbf_sb[:, 0:ncols], start=True, stop=True)

        # issue ALL x slab loads now, in consumption order, on the SP ring
        xt_sbs = []  # per unit: (tile, col offset)
        for dram, n_units in ((x0, 1), (x1, 1), (x23, 2), (x45, 2),
                              (x6, 1), (x7, 1)):
            xt_sb = xt_pool.tile([128, n_units * 4 * UB], BF16,
                                 name="xt", tag="xt")
            nc.sync.dma_start(xt_sb, dram[:, :])
            for j in range(n_units):
                xt_sbs.append((xt_sb, j * 4 * UB))

        # pre-warm the PE while the first x slab is still in flight
        emit_fill(18, 128)

        # per-unit tiles kept for cross-stage use (unit = 1024 samples)
        th_bf = [None] * UNITS
        psi = [None] * UNITS

        def emit_pre(u):
            """pre-net matmuls + bias + tanh for unit u, sample-major.

            x tile blocks are the STATIONARY operand (128-col bf16
            LDWEIGHTS -> automatic fast-weight-load) and the tiny pre_w
            chunk streams as the 4-column moving operand, so pre_out
            lands sample-major [128, 4] per tile and no transpose back
            from feature-major is ever needed.  The per-feature (free-dim)
            bias cannot be an activation bias (per-partition only) and a
            K=1 accumulating matmul clobbers PSUM, so it is a DVE add
            against a broadcast const tile on the way out of PSUM.
            """
            xt_sb, off = xt_sbs[u]
            po = ps_po.tile([128, 32], F32, name="po", tag="po")
            for t in range(8):
                for k in range(4):
                    c0 = off + UB * k + 128 * t
                    nc.tensor.matmul(
                        po[:, 4 * t:4 * (t + 1)],
                        lhsT=xt_sb[:, c0:c0 + 128],
                        rhs=pre_wt_sb[:, 4 * k:4 * (k + 1)],
                        start=(k == 0), stop=(k == 3))
            pre2 = work.tile([128, 32], F32, name="pre2", tag="pre2")
            nc.vector.tensor_tensor(out=pre2, in0=po, in1=preb32_sb,
                                    op=mybir.AluOpType.add)
            th = work.tile([128, 32], BF16, name="th_bf", tag="th_bf")
            nc.scalar.activation(th, pre2, AF.Tanh)
            th_bf[u] = th

        def emit_trig(u):
            """trig + psi products for unit u (all sample-major)."""
            th = th_bf[u]
            cs = work.tile([128, 64], F32, name="cs", tag="cs")
            cs4 = cs[:, :].rearrange("p (t w x) -> p t w x", w=4, x=2)
            th3 = th[:, :].rearrange("p (t w) -> p t w", w=4)
            # cos(theta) = sin(pi/4*tanh + 3pi/4); sin(theta) = sin(.. + pi/4)
            nc.scalar.activation(cs4[:, :, :, 0], th3, AF.Sin,
                                 bias=trigb_sb[:, 0:1], scale=PI4)
            nc.scalar.activation(cs4[:, :, :, 1], th3, AF.Sin,
                                 bias=trigb_sb[:, 1:2], scale=PI4)
            cs8 = cs[:, :].rearrange("p (t w) -> p t w", w=8)
            v01 = work.tile([128, 32], F32, name="v01", tag="v01")
            v23 = work.tile([128, 32], F32, name="v23", tag="v23")
            nc.vector.tensor_tensor(
                out=v01[:, :].rearrange("p (t a b) -> p t a b", a=2, b=2),
                in0=cs8[:, :, 0:2].unsqueeze(3).broadcast_to((128, 8, 2, 2)),
                in1=cs8[:, :, 2:4].unsqueeze(2).broadcast_to((128, 8, 2, 2)),
                op=mybir.AluOpType.mult)
            nc.vector.tensor_tensor(
                out=v23[:, :].rearrange("p (t a b) -> p t a b", a=2, b=2),
                in0=cs8[:, :, 4:6].unsqueeze(3).broadcast_to((128, 8, 2, 2)),
                in1=cs8[:, :, 6:8].unsqueeze(2).broadcast_to((128, 8, 2, 2)),
                op=mybir.AluOpType.mult)
            ps = work.tile([128, 128], F32, name="psi", tag="psi")
            nc.vector.tensor_tensor(
                out=ps[:, :].rearrange("p (t a b) -> p t a b", a=4, b=4),
                in0=v01[:, :].rearrange("p (t i) -> p t i", i=4)
                    .unsqueeze(3).broadcast_to((128, 8, 4, 4)),
                in1=v23[:, :].rearrange("p (t i) -> p t i", i=4)
                    .unsqueeze(2).broadcast_to((128, 8, 4, 4)),
                op=mybir.AluOpType.mult)
            psi[u] = ps

        def emit_quantum(u):
            """PE transpose, M matmul, square, P matmul, +post_b, store
            for unit u."""
            psiT_ps = ps2.tile([128, 128], F32, name="psiT_ps", tag="p2")
            nc.tensor.transpose(psiT_ps, psi[u][:, :], id_sb)
            pt = work.tile([128, 128], BF16, name="psiT", tag="psiT")
            nc.vector.tensor_copy(pt, psiT_ps)  # f32 psum -> bf16 sbuf
            phiT_ps = ps2.tile([128, 128], F32, name="phiT_ps", tag="p2")
            nc.tensor.matmul(phiT_ps, lhsT=mbd_sb, rhs=pt,
                             start=True, stop=True)
            ph = work.tile([128, 128], BF16, name="phi2", tag="phi2")
            # DVE cannot read two non-scalar PSUM inputs -> Square on ScalarE
            nc.scalar.activation(ph, phiT_ps, AF.Square)
            o10_ps = ps2.tile([80, 128], F32, name="o10_ps", tag="p2")
            nc.tensor.matmul(o10_ps, lhsT=pbd_sb, rhs=ph,
                             start=True, stop=True)
            # +post_b on the DVE, landing in the transposed output staging
            nc.vector.tensor_tensor(
                out=out2_sb[:, 128 * u:128 * (u + 1)],
                in0=o10_ps, in1=pb80_sb.broadcast_to((80, 128)),
                op=mybir.AluOpType.add)
            # store this unit's [8 tiles, 10 classes, 128 samples] now; the
            # sim-time hint keeps its semaphore lane behind the early loads
            with tc.tile_wait_until(0.05):
                nc.sync.dma_start(
                    out[8 * u:8 * (u + 1), :, :]
                        .rearrange("t c p -> (t c) p"),
                    out2_sb[:, 128 * u:128 * (u + 1)])

        # software-pipelined emission: trig lags pre by 1 unit, quantum by
        # 2, so no engine FIFO head-blocks on a cross-engine dep that is
        # not ready; fillers keep the PE's activity monitor hot in the gaps
        for u in range(UNITS):
            emit_pre(u)
            if u < UNITS - 1:
                emit_fill(4)
            if u >= 1:
                emit_trig(u - 1)
            if u >= 2:
                emit_quantum(u - 2)
        emit_trig(UNITS - 1)
        emit_quantum(UNITS - 2)
        emit_quantum(UNITS - 1)

    nc.finalize()  # bacc: register alloc + event-semaphore wait splitting
    return nc


_NC_CACHE: dict = {}


def _get_nc() -> bass.Bass:
    if "nc" not in _NC_CACHE:
        _NC_CACHE["nc"] = build_nc()
    return _NC_CACHE["nc"]


def make_in_maps(inputs: dict) -> list:
    x = np.asarray(inputs["input_features"], np.float32)
    pre_w = np.asarray(inputs["pre_w"], np.float32)
    pre_b = np.asarray(inputs["pre_b"], np.float32)
    q_params = np.asarray(inputs["q_params"], np.float32)
    post_w = np.asarray(inputs["post_w"], np.float32)
    post_b = np.asarray(inputs["post_b"], np.float32)

    M = _build_M(q_params)
    P = _build_P(post_w)
    cbf = np.zeros((128, CBF_COLS), np.float32)
    # pre_wt[p, 4k+f] = pre_w[f, 128k+p]
    cbf[:, PW0:PW0 + 16] = (
        pre_w.T.reshape(4, 128, 4).transpose(1, 0, 2).reshape(128, 16))
    for t in range(8):
        cbf[16 * t:16 * (t + 1), MB0 + 16 * t:MB0 + 16 * (t + 1)] = M.T
        cbf[16 * t:16 * (t + 1), PB0 + 10 * t:PB0 + 10 * (t + 1)] = P
    cbf = cbf.astype(ml_dtypes.bfloat16)

    cf32 = np.zeros((128, CF32_COLS), np.float32)
    cf32[0:80, 0] = np.tile(post_b, 8)
    cf32[:, 1] = 3.0 * PI4
    cf32[:, 2] = PI4
    cf32[:, 3:131] = np.eye(128)
    cf32[:, 131:163] = np.tile(pre_b, 8)[None, :]

    xb = x.astype(ml_dtypes.bfloat16)
    consts = dict(cbf=cbf, cf32=cf32)
    in_maps = []
    for i in range(N_CORES):
        # xt[u][p, 1024k + 128t + b] = x_core[1024u + 128t + b, 128k + p]
        xc = xb[B * i:B * (i + 1)]                       # [8192, 512]
        xct = np.ascontiguousarray(
            xc.reshape(UNITS, 8, 128, 4, 128).transpose(0, 4, 3, 1, 2)
        ).reshape(UNITS, 128, 4 * UB)
        in_maps.append(dict(
            x0=xct[0], x1=xct[1],
            x23=np.concatenate([xct[2], xct[3]], axis=1),
            x45=np.concatenate([xct[4], xct[5]], axis=1),
            x6=xct[6], x7=xct[7],
            **consts))
    return in_maps


def unpack_out(dev_out: np.ndarray) -> np.ndarray:
    """[TILES, C, 128] device layout -> [B, C]."""
    return dev_out.transpose(0, 2, 1).reshape(B, C)


def run_on_device(inputs: dict, **kwargs):
    """Returns (full_output, BassKernelResults)."""
    nc = _get_nc()
    in_maps = make_in_maps(inputs)
    res = run_bass_kernel_spmd(nc, in_maps, core_ids=list(range(N_CORES)),
                               **kwargs)
    full = np.concatenate(
        [unpack_out(res.results[i]["out"]) for i in range(N_CORES)], 0)
    return np.ascontiguousarray(full, dtype=np.float32), res


def kernel(**inputs) -> np.ndarray:
    out, _ = run_on_device(inputs)
    return out
